# revision 3
# baseline (speedup 1.0000x reference)
"""Trainium2 Bass kernel for: conv2d(16->64, 3x3, VALID) + bias -> min over
channels -> tanh(tanh()).  Input x [64,16,256,256] f32, output [64,1,254,254].

Strategy (per core, data-parallel over batch: 8 images/core):
  - Conv as matmuls with the x-patch as the stationary operand and a
    block-Toeplitz weight matrix moving, so conv output lands as
    [width-positions (partitions), rows*couts (free)] and the channel-min is
    a free-dim DVE reduce_min.
  - Contraction K = 17 channels x 7 rows = 119 (channel 16 is a host-added
    ones-channel carrying the bias).  A 7-row window yields R=5 output rows
    per matmul trio (3 PSUM-accumulated matmuls, one per kernel x-tap).
  - PSUM is organized as two 4-bank tiles [128, 4, 512] (all 8 banks).  The
    PE fills 4 window-trios into one tile while the DVE runs ONE grouped
    tensor_reduce(min) over the other ([127, 4, 5, 64] -> [127, 20]),
    amortizing the per-op PSUM access penalty 4x so the DVE keeps up with
    the PE and the PE never stalls.
  - Min results go to a bf16 staging tile [128 j, 256 rows]; epilogue uses
    the XBAR dma_start_transpose (DMA engines, not PE) to flip to
    [row, j] layout, double-tanh on ScalarE, contiguous row stores.
"""

import sys

for _p in ("/opt/trn_rl_repo", "/root/.axon_site/_ro/trn_rl_repo"):
    if _p not in sys.path:
        sys.path.insert(0, _p)

import numpy as np

B, CIN, H, W = 64, 16, 256, 256
COUT, KK = 64, 3
HO, WO = H - 2, W - 2  # 254
N_CORES = 8
B_LOC = B // N_CORES  # 8 images per core

# geometry
WIN_ROWS = 7          # input rows per window
R = WIN_ROWS - KK + 1  # 5 output rows per window
KDIM = (CIN + 1) * WIN_ROWS  # 119 contraction rows (incl. ones channel)
NDIM = R * COUT       # 320 moving free size
MJ = 127              # output width positions per j-block
N_JB = 2              # j blocks (2*127 = 254)
N_WIN = 51            # windows: row0 = 5w for w<50, 249 for w=50
WIN_COLS = MJ + KK - 1  # 129 columns per window tile
BANK = 512            # psum bank, f32 elements per partition
GROUPS = [(w0, 4) for w0 in range(0, 48, 4)] + [(48, 2), (50, 1)]

_cache = {}


def _build_wblocks(conv_weight, conv_bias):
    """wblk[dx][rho*17+ci, r*64+co] = W[co,ci,rho-r,dx]; bias on the ones-
    channel row (rho=0, ci=CIN) of dx=0.  Partition order matches the
    [B, H, C, W] host layout of x so the window DMA merges (row, chan)."""
    wblk = np.zeros((KK, KDIM, NDIM), dtype=np.float32)
    for dx in range(KK):
        for ci in range(CIN):
            for rho in range(WIN_ROWS):
                k = rho * (CIN + 1) + ci
                for r in range(R):
                    dy = rho - r
                    if 0 <= dy < KK:
                        wblk[dx, k, r * COUT:(r + 1) * COUT] = conv_weight[:, ci, dy, dx]
    k_bias = CIN  # (rho=0, ci=16)
    for r in range(R):
        wblk[0, k_bias, r * COUT:(r + 1) * COUT] = conv_bias
    return wblk


def _build_nc(reps=1, ablate=()):
    import concourse.bass as bass
    import concourse.bacc as bacc
    import concourse.tile as tile
    from concourse import mybir

    f32 = mybir.dt.float32
    f32r = mybir.dt.float32r

    nc = bacc.Bacc(None)
    # x_aug host layout is [B, H, C, W]: window partitions are (row, chan)
    x_aug = nc.dram_tensor("x_aug", [B_LOC, H, CIN + 1, W], f32r, kind="ExternalInput")
    wblk_d = nc.dram_tensor("wblk", [KK, KDIM, NDIM], f32r, kind="ExternalInput")
    y = nc.dram_tensor("y", [B_LOC, HO, WO], f32, kind="ExternalOutput")

    with tile.TileContext(nc) as tc:
        with (
            tc.tile_pool(name="consts", bufs=1) as consts,
            tc.tile_pool(name="wins", bufs=3) as wins,
            tc.tile_pool(name="stage", bufs=4) as stage,
            tc.tile_pool(name="outs", bufs=4) as outs,
            tc.tile_pool(name="cpsum", bufs=2, space="PSUM") as cpsum,
        ):
            wblk_s = consts.tile([KDIM, KK, NDIM], f32r)
            nc.sync.dma_start(out=wblk_s[:], in_=wblk_d.rearrange("k d n -> d k n"))

            import contextlib
            loop_ctx = tc.For_i(0, reps, 1) if reps > 1 else contextlib.nullcontext()
            with loop_ctx:
                _emit_body(nc, tc, bass, mybir, ablate, locals())
    nc.finalize()
    return nc


def _emit_body(nc, tc, bass, mybir, ablate, env):
    f32 = env["f32"]
    f32r = env["f32r"]
    bf16 = mybir.dt.bfloat16
    x_aug, y = env["x_aug"], env["y"]
    wblk_s = env["wblk_s"]
    wins, stage, outs = env["wins"], env["stage"], env["outs"]
    cpsum = env["cpsum"]
    CW = (CIN + 1) * W  # elements per image row (all channels)

    for b in range(B_LOC):
        bigx = wins.tile([KDIM, N_WIN, W], f32r, name="bigx")
        if "nodma" not in ablate:
            # windows 0..49 (uniform row0 = 5w) in 4 chunked DMAs; w=50 alone
            x_b = x_aug[b]
            for ci, w_lo in enumerate(range(0, 50, 13)):
                w_hi = min(w_lo + 13, 50)
                nw = w_hi - w_lo
                src = bass.AP(
                    tensor=x_b.tensor,
                    offset=x_b.offset + 5 * w_lo * CW,
                    ap=[[CW, WIN_ROWS], [W, CIN + 1], [5 * CW, nw], [1, W]],
                )
                nc.sync.dma_start(out=bigx[:, w_lo:w_hi, :], in_=src)
            nc.sync.dma_start(
                out=bigx[:, N_WIN - 1, :],
                in_=x_aug[b, HO - R:H, :, :].rearrange("r c w -> (r c) w"),
            )
        for jb in range(N_JB):
            j0 = jb * MJ
            staging = stage.tile([128, 256], bf16, name=f"staging{jb}", tag=f"st{jb}")
            for w0, gn in GROUPS:
                psum = cpsum.tile([128, 4, BANK], f32, name="psum")
                if "nomm" not in ablate:
                    for g in range(gn):
                        w = w0 + g
                        for dx in range(KK):
                            nc.tensor.matmul(
                                out=psum[0:MJ, g, 0:NDIM],
                                lhsT=bigx[:, w, j0 + dx:j0 + dx + MJ],
                                rhs=wblk_s[:, dx, :],
                                start=(dx == 0),
                                stop=(dx == KK - 1),
                            )
                if "nodve" not in ablate:
                    row0 = 5 * w0 if w0 < N_WIN - 1 else HO - R
                    rows = 5 * gn
                    nc.vector.tensor_reduce(
                        out=staging[0:MJ, row0:row0 + rows].rearrange(
                            "p (g r) -> p g r", g=gn),
                        in_=psum[0:MJ, 0:gn, 0:NDIM].rearrange(
                            "p g (r c) -> p g r c", c=COUT),
                        axis=mybir.AxisListType.X,
                        op=mybir.AluOpType.min,
                    )
            if "noepi" in ablate:
                continue
            # epilogue: XBAR transpose (DMA) -> double tanh (ACT) -> store
            tTs = []
            for k in range(2):
                tT = outs.tile([128, 128], bf16, name="tT")
                nc.scalar.dma_start_transpose(
                    out=tT[:], in_=staging[:, 128 * k:128 * (k + 1)])
                tTs.append(tT)
            for k in range(2):
                t1 = outs.tile([128, 128], f32, name="t1")
                nc.scalar.activation(
                    out=t1[:], in_=tTs[k][:],
                    func=mybir.ActivationFunctionType.Tanh,
                )
                t2 = outs.tile([128, 128], f32, name="t2")
                nc.scalar.activation(
                    out=t2[:], in_=t1[:],
                    func=mybir.ActivationFunctionType.Tanh,
                )
                nrows = 128 if k == 0 else HO - 128  # 126 valid rows in k=1
                nc.scalar.dma_start(
                    out=y[b, 128 * k:128 * k + nrows, j0:j0 + MJ],
                    in_=t2[0:nrows, 0:MJ],
                )


def _get_compiled(reps=1, ablate=()):
    key = ("nc", reps, tuple(ablate))
    if key not in _cache:
        _cache[key] = _build_nc(reps, ablate)
    return _cache[key]


def kernel(x, conv_weight, conv_bias):
    from concourse.bass_utils import run_bass_kernel_spmd

    x = np.asarray(x, dtype=np.float32)
    conv_weight = np.asarray(conv_weight, dtype=np.float32)
    conv_bias = np.asarray(conv_bias, dtype=np.float32)

    x_aug = np.empty((B, H, CIN + 1, W), dtype=np.float32)
    x_aug[:, :, :CIN] = x.transpose(0, 2, 1, 3)
    x_aug[:, :, CIN] = 1.0
    wblk = _build_wblocks(conv_weight, conv_bias)

    nc = _get_compiled()
    in_maps = [
        {
            "x_aug": np.ascontiguousarray(x_aug[c * B_LOC:(c + 1) * B_LOC]),
            "wblk": wblk,
        }
        for c in range(N_CORES)
    ]
    res = run_bass_kernel_spmd(nc, in_maps, core_ids=list(range(N_CORES)))
    out = np.concatenate([res.results[c]["y"] for c in range(N_CORES)], axis=0)
    return out.reshape(B, 1, HO, WO)


# revision 15
# speedup vs baseline: 1.1525x; 1.1525x over previous
"""Trainium2 Bass kernel for: conv2d(16->64, 3x3, VALID) + bias -> min over
channels -> tanh(tanh()).  Input x [64,16,256,256] f32, output [64,1,254,254].

Strategy (per core, data-parallel over batch: 8 images/core):
  - Conv as matmuls with the x-patch (bf16) as the stationary operand and a
    block-Toeplitz weight matrix (bf16) moving, so conv output lands as
    [width-positions (partitions), rows*couts (free)] in f32 PSUM and the
    channel-min is a free-dim reduce.  Contraction K = 17 channels x 7 rows
    = 119 (channel 16 is a host-added ones-channel carrying the bias); a
    7-row window yields R=5 output rows per 3-matmul (dx) PSUM trio.
    bf16 + 128-wide stationary enables Fast Weight Load and halves the
    input DMA (x error ~2^-9 << the 2e-2 gate).
  - The DVE tensor_reduce(min) is capped at 1 elem/cycle (no 2x/4x uop) and
    alone cannot keep up with the PE (458ns/tile vs ~400ns production), so
    ~40% of the (w, jb) tiles are consumed by an offload chain instead:
    ScalarE copies PSUM->SBUF as bf16 (fold-friendly layout), GPSIMD does
    two tensor-tensor min folds (64->32->16 couts), DVE finishes with a
    cheap [*,5,16] reduce.  All engines stay under ~80% so the PE never
    stalls.
  - Min results land in bf16 staging [128 j, 256 rows]; epilogue: PE
    transpose (identity matmul), double-tanh on ScalarE from PSUM,
    contiguous row stores on the ACT HWDGE ring.
"""

import sys

for _p in ("/opt/trn_rl_repo", "/root/.axon_site/_ro/trn_rl_repo"):
    if _p not in sys.path:
        sys.path.insert(0, _p)

import numpy as np

B, CIN, H, W = 64, 16, 256, 256
COUT, KK = 64, 3
HO, WO = H - 2, W - 2  # 254
N_CORES = 8
B_LOC = B // N_CORES  # 8 images per core

# geometry
WIN_ROWS = 7          # input rows per window
R = WIN_ROWS - KK + 1  # 5 output rows per window
KDIM = (CIN + 1) * WIN_ROWS  # 119 contraction rows (incl. ones channel)
NDIM = R * COUT       # 320 moving free size
MJ = 128              # output width positions per j-block
J0S = (0, WO - MJ)    # j origin per block; cols 126/127 overlap benignly
N_JB = 2
N_WIN = 51            # windows: row0 = 5w for w<50, 249 for w=50
_cache = {}


def _is_b(w, jb):
    """Offload-chain tiles: ~half, never the last windows (keeps the
    w=49/50 staging-row-249 WAW on one engine)."""
    return w < 48 and (w + jb) % 2 == 1


def _build_wblocks(conv_weight, conv_bias):
    """wblk[dx][rho*17+ci, r*64+co] = W[co,ci,rho-r,dx]; bias on the ones-
    channel row (rho=0, ci=CIN) of dx=0.  Partition order matches the
    [B, H, C, W] host layout of x so the window DMA merges (row, chan)."""
    wblk = np.zeros((KK, KDIM, NDIM), dtype=np.float32)
    for dx in range(KK):
        for ci in range(CIN):
            for rho in range(WIN_ROWS):
                k = rho * (CIN + 1) + ci
                for r in range(R):
                    dy = rho - r
                    if 0 <= dy < KK:
                        wblk[dx, k, r * COUT:(r + 1) * COUT] = conv_weight[:, ci, dy, dx]
    k_bias = CIN  # (rho=0, ci=16)
    for r in range(R):
        wblk[0, k_bias, r * COUT:(r + 1) * COUT] = conv_bias
    return wblk


def _build_nc(reps=1, ablate=()):
    import concourse.bass as bass
    import concourse.bacc as bacc
    import concourse.tile as tile
    from concourse import mybir

    f32 = mybir.dt.float32
    bf16 = mybir.dt.bfloat16

    nc = bacc.Bacc(None)
    # x_aug host layout is [B, H, C, W]: window partitions are (row, chan)
    x_aug = nc.dram_tensor("x_aug", [B_LOC, H, CIN + 1, W], bf16, kind="ExternalInput")
    wblk_d = nc.dram_tensor("wblk", [KK, KDIM, NDIM], bf16, kind="ExternalInput")
    ident_d = nc.dram_tensor("ident", [MJ, MJ], bf16, kind="ExternalInput")
    y = nc.dram_tensor("y", [B_LOC, HO, WO], f32, kind="ExternalOutput")

    with tile.TileContext(nc) as tc:
        with (
            tc.tile_pool(name="consts", bufs=1) as consts,
            tc.tile_pool(name="wins", bufs=3) as wins,
            tc.tile_pool(name="stage", bufs=4) as stage,
            tc.tile_pool(name="fold", bufs=4) as fold,
            tc.tile_pool(name="outs", bufs=4) as outs,
            tc.tile_pool(name="cpsum", bufs=6, space="PSUM") as cpsum,
            tc.tile_pool(name="tpsum", bufs=2, space="PSUM") as tpsum,
        ):
            wblk_s = consts.tile([KDIM, KK, NDIM], bf16)
            nc.sync.dma_start(out=wblk_s[:], in_=wblk_d.rearrange("k d n -> d k n"))
            ident_s = consts.tile([MJ, MJ], bf16)
            nc.sync.dma_start(out=ident_s[:], in_=ident_d[:])

            import contextlib
            loop_ctx = tc.For_i(0, reps, 1) if reps > 1 else contextlib.nullcontext()
            with loop_ctx:
                _emit_body(nc, tc, bass, mybir, ablate, locals())
    nc.finalize()
    return nc


def _emit_body(nc, tc, bass, mybir, ablate, env):
    f32 = env["f32"]
    bf16 = env["bf16"]
    x_aug, y = env["x_aug"], env["y"]
    wblk_s, ident_s = env["wblk_s"], env["ident_s"]
    wins, stage, fold, outs = env["wins"], env["stage"], env["fold"], env["outs"]
    cpsum, tpsum = env["cpsum"], env["tpsum"]
    CW = (CIN + 1) * W  # elements per image row (all channels)
    MIN = mybir.AluOpType.min

    for b in range(B_LOC):
        bigx = wins.tile([KDIM, N_WIN, W], bf16, name="bigx")
        if "nodma" in ablate:
            nc.sync.dma_start(
                out=bigx[:, 0, :],
                in_=x_aug[b, 0:WIN_ROWS, :, :].rearrange("r c w -> (r c) w"),
            )
        else:
            # windows 0..49 (uniform row0 = 5w) in 2 chunked DMAs; w=50 alone
            x_b = x_aug[b]
            for w_lo in (0, 25):
                src = bass.AP(
                    tensor=x_b.tensor,
                    offset=x_b.offset + 5 * w_lo * CW,
                    ap=[[CW, WIN_ROWS], [W, CIN + 1], [5 * CW, 25], [1, W]],
                )
                nc.sync.dma_start(out=bigx[:, w_lo:w_lo + 25, :], in_=src)
            nc.sync.dma_start(
                out=bigx[:, N_WIN - 1, :],
                in_=x_aug[b, HO - R:H, :, :].rearrange("r c w -> (r c) w"),
            )
        stagings = []
        for jb in range(N_JB):
            staging = stage.tile([MJ, 256], bf16, name=f"staging{jb}", tag=f"st{jb}")
            stagings.append(staging)
        for w in range(N_WIN):
            row0 = 5 * w if w < N_WIN - 1 else HO - R
            for jb in range(N_JB):
                j0 = J0S[jb]
                psum = cpsum.tile([MJ, NDIM], f32, name="psum")
                if "nomm" not in ablate:
                    wi = 0 if "nodma" in ablate else w
                    for dx in range(KK):
                        nc.tensor.matmul(
                            out=psum[:],
                            lhsT=bigx[:, wi, j0 + dx:j0 + dx + MJ],
                            rhs=wblk_s[:, dx, :],
                            start=(dx == 0),
                            stop=(dx == KK - 1),
                        )
                if "nodve" in ablate:
                    continue
                if _is_b(w, jb) and "nooff" not in ablate:
                    # offload chain: ACT cast-copy to bf16, then a packed
                    # 2x-mode DVE fold (tensor_tensor min has a 2x_1p uop;
                    # tensor_reduce is 1x-only) halves the reduce input
                    sb = fold.tile([MJ, 2, R, 32], bf16, name="sb")
                    nc.scalar.activation(
                        out=sb.rearrange("p c2 r c -> p r c2 c"),
                        in_=psum.rearrange("p (r c2 c) -> p r c2 c", c2=2, c=32),
                        func=mybir.ActivationFunctionType.Copy,
                    )
                    f1 = fold.tile([MJ, R, 32], bf16, name="f1")
                    nc.vector.tensor_tensor(
                        out=f1[:], in0=sb[:, 0], in1=sb[:, 1], op=MIN,
                    )
                    nc.vector.tensor_reduce(
                        out=stagings[jb][:, row0:row0 + R],
                        in_=f1[:],
                        axis=mybir.AxisListType.X,
                        op=MIN,
                    )
                else:
                    nc.vector.tensor_reduce(
                        out=stagings[jb][:, row0:row0 + R],
                        in_=psum.rearrange("p (r c) -> p r c", c=COUT),
                        axis=mybir.AxisListType.X,
                        op=MIN,
                    )
        if "noepi" in ablate:
            continue
        for jb in range(N_JB):
            j0 = J0S[jb]
            for rb in range(2):
                ps_t = tpsum.tile([MJ, MJ], bf16, name="ps_t")
                nc.tensor.transpose(
                    out=ps_t[:], in_=stagings[jb][:, 128 * rb:128 * rb + MJ],
                    identity=ident_s[:],
                )
                t1 = outs.tile([MJ, MJ], f32, name="t1")
                nc.scalar.activation(
                    out=t1[:], in_=ps_t[:],
                    func=mybir.ActivationFunctionType.Tanh,
                )
                t2 = outs.tile([MJ, MJ], f32, name="t2")
                nc.scalar.activation(
                    out=t2[:], in_=t1[:],
                    func=mybir.ActivationFunctionType.Tanh,
                )
                nrows = 128 if rb == 0 else HO - 128  # 126 valid rows in rb=1
                nc.scalar.dma_start(
                    out=y[b, 128 * rb:128 * rb + nrows, j0:j0 + MJ],
                    in_=t2[0:nrows, 0:MJ],
                )


def _get_compiled(reps=1, ablate=()):
    key = ("nc", reps, tuple(ablate))
    if key not in _cache:
        _cache[key] = _build_nc(reps, ablate)
    return _cache[key]


def _to_bf16(a):
    import ml_dtypes
    return np.asarray(a, dtype=np.float32).astype(ml_dtypes.bfloat16)


def make_in_maps(x, conv_weight, conv_bias):
    x = np.asarray(x, dtype=np.float32)
    x_aug = np.empty((B, H, CIN + 1, W), dtype=np.float32)
    x_aug[:, :, :CIN] = x.transpose(0, 2, 1, 3)
    x_aug[:, :, CIN] = 1.0
    x_aug = _to_bf16(x_aug)
    wblk = _to_bf16(_build_wblocks(
        np.asarray(conv_weight, dtype=np.float32),
        np.asarray(conv_bias, dtype=np.float32)))
    ident = _to_bf16(np.eye(MJ, dtype=np.float32))
    return [
        {
            "x_aug": np.ascontiguousarray(x_aug[c * B_LOC:(c + 1) * B_LOC]),
            "wblk": wblk,
            "ident": ident,
        }
        for c in range(N_CORES)
    ]


def kernel(x, conv_weight, conv_bias):
    from concourse.bass_utils import run_bass_kernel_spmd

    nc = _get_compiled()
    in_maps = make_in_maps(x, conv_weight, conv_bias)
    res = run_bass_kernel_spmd(nc, in_maps, core_ids=list(range(N_CORES)))
    out = np.concatenate([res.results[c]["y"] for c in range(N_CORES)], axis=0)
    return out.reshape(B, 1, HO, WO)
